# revision 2
# baseline (speedup 1.0000x reference)
"""Causal self-attention (B=4, T=2048, C=1024, nh=16) on 8 Trainium2 NeuronCores.

Sharding: tensor-parallel over heads (2 heads/core). Each core receives:
  - xT:  [1024, 8192]  transposed activations (replicated)
  - wq:  [1024, 384]   Wqkv columns for its 2 heads, ordered [q0|q1|k0|k1|v0|v1]
  - wp:  [128, 1024]   Wproj rows for its 2 heads' channels
and produces y_part (full-shape partial, host-summed), plus its kT/vT shards.

Device pipeline per batch b:
  A) qkvT = wq_shard.T @ xT  -> Q^T,K^T,V^T stacked [128, 2048] (f32r)
  B) V~ = PE-transpose(V^T) padded to [128(k),128] with a ones column (rowsum trick)
  C) per 512-q block j: S^T[k,q] = K^T_stat.T @ Q^T_mov; causal mask on diagonal
     tiles; P^T = exp(S^T/8) (ACT, f32r); O_aug^T[65,q] += V~^T @ P^T
  D) transpose O_aug -> divide rows by rowsum col -> transpose back -> O^T
  E) y_part[t, :] = O^T_stat.T @ Wproj_mov  (single 128-contraction)

All matmuls run in float32r (1 cyc/row at N=512; ~1.6e-4 rel err).
"""

import numpy as np
from contextlib import ExitStack

import concourse.bass as bass
import concourse.tile as tile
from concourse import bacc, mybir
from concourse.bass_utils import run_bass_kernel_spmd
from concourse.masks import make_identity

F32 = mybir.dt.float32
F32R = mybir.dt.float32r
EXP = mybir.ActivationFunctionType.Exp

N_CORES = 8
B, T, C = 4, 2048, 1024
NH, HD = 16, 64
HPC = NH // N_CORES          # heads per core = 2
BT = B * T                   # 8192

_CACHE = {}


def _build():
    nc = bacc.Bacc("TRN2", target_bir_lowering=False, debug=False, num_devices=N_CORES)
    xT = nc.dram_tensor("xT", [C, BT], F32, kind="ExternalInput").ap()
    wq = nc.dram_tensor("wq", [C, 3 * HPC * HD], F32, kind="ExternalInput").ap()
    wp = nc.dram_tensor("wp", [HPC * HD, C], F32, kind="ExternalInput").ap()
    y_out = nc.dram_tensor("y_part", [BT, C], F32, kind="ExternalOutput").ap()
    kT_out = nc.dram_tensor("kT", [B, HPC * HD, T], F32, kind="ExternalOutput").ap()
    vT_out = nc.dram_tensor("vT", [B, HPC * HD, T], F32, kind="ExternalOutput").ap()

    with tile.TileContext(nc) as tc, ExitStack() as ctx:
        const = ctx.enter_context(tc.tile_pool(name="const", bufs=1))
        bpool = ctx.enter_context(tc.tile_pool(name="bpool", bufs=2))
        xpool = ctx.enter_context(tc.tile_pool(name="xpool", bufs=2))
        ppool = ctx.enter_context(tc.tile_pool(name="ppool", bufs=4))
        dpool = ctx.enter_context(tc.tile_pool(name="dpool", bufs=2))
        npool = ctx.enter_context(tc.tile_pool(name="npool", bufs=8))
        spool = ctx.enter_context(tc.tile_pool(name="spool", bufs=8))
        ypool = ctx.enter_context(tc.tile_pool(name="ypool", bufs=3))
        psA = ctx.enter_context(tc.tile_pool(name="psA", bufs=2, space="PSUM"))
        psS = ctx.enter_context(tc.tile_pool(name="psS", bufs=3, space="PSUM"))
        psO = ctx.enter_context(tc.tile_pool(name="psO", bufs=3, space="PSUM"))

        # --- constants ---
        ident = const.tile([128, 128], F32)
        make_identity(nc, ident[:])
        identr = const.tile([128, 128], F32R)
        nc.scalar.copy(out=identr[:], in_=ident[:])
        # additive causal masks for the 4 diagonal offsets of S^T [k=128, q=512]
        masks = const.tile([128, 4, 512], F32)
        for o in range(4):
            nc.gpsimd.memset(masks[:, o, :], 0.0)
            nc.gpsimd.affine_select(
                out=masks[:, o, :], in_=masks[:, o, :],
                compare_op=mybir.AluOpType.is_ge,
                fill=-1e10, base=-128 * o,
                pattern=[[1, 512]], channel_multiplier=-1,
            )
        # pad block for V~ columns 64..127: col 64 = 1 (rowsum), rest 0
        padcol = const.tile([128, 64], F32)
        nc.any.memset(padcol[:], 0.0)
        nc.any.memset(padcol[:, 0:1], 1.0)
        # weights (DMA straight into f32r tiles)
        wq_sb = const.tile([128, 8, 3 * 128], F32R)
        for kc in range(8):
            nc.sync.dma_start(out=wq_sb[:, kc, :], in_=wq[kc * 128:(kc + 1) * 128, :].bitcast(F32R))
        wp_sb = const.tile([128, C], F32R)
        nc.sync.dma_start(out=wp_sb[:], in_=wp[:].bitcast(F32R))

        for b in range(B):
            QT = bpool.tile([128, T], F32R, tag="QT")
            KT = bpool.tile([128, T], F32R, tag="KT")
            VT = bpool.tile([128, T], F32R, tag="VT")
            OT = bpool.tile([128, T], F32R, tag="OT")
            vtil = bpool.tile([128, HPC, 16, 128], F32R, tag="vtil")

            # --- A: qkvT for this batch ---
            for tb in range(4):
                t0 = b * T + tb * 512
                xt = xpool.tile([128, 8, 512], F32R, tag="xt")
                for kc in range(8):
                    nc.sync.dma_start(
                        out=xt[:, kc, :],
                        in_=xT[kc * 128:(kc + 1) * 128, t0:t0 + 512].bitcast(F32R),
                    )
                for g, dest in enumerate((QT, KT, VT)):
                    pq = psA.tile([128, 512], F32, tag="a")
                    for kc in range(8):
                        nc.tensor.matmul(
                            pq[:], wq_sb[:, kc, g * 128:(g + 1) * 128], xt[:, kc, :],
                            start=(kc == 0), stop=(kc == 7),
                        )
                    nc.any.tensor_copy(out=dest[:, tb * 512:tb * 512 + 512], in_=pq[:])
            nc.sync.dma_start(out=kT_out[b], in_=KT[:].bitcast(F32))
            nc.sync.dma_start(out=vT_out[b], in_=VT[:].bitcast(F32))

            # --- B: V~ (V natural + ones column, padded to 128) ---
            for tc_ in range(16):
                pt = psA.tile([128, 128], F32R, tag="a")
                nc.tensor.transpose(pt[:], VT[:, tc_ * 128:(tc_ + 1) * 128], identr[:])
                for h in range(HPC):
                    nc.any.tensor_copy(out=vtil[:, h, tc_, 0:64], in_=pt[:, h * 64:h * 64 + 64])
                    nc.scalar.copy(out=vtil[:, h, tc_, 64:128], in_=padcol[:])

            # --- C: attention per 512-q block ---
            for j in range(4):
                oaug = [psO.tile([128, 512], F32, tag="o", name=f"oaug{b}_{j}_{h}") for h in range(HPC)]
                nkc = 4 * j + 4
                for kc in range(nkc):
                    for h in range(HPC):
                        sps = psS.tile([128, 512], F32, tag="s")
                        nc.tensor.matmul(
                            sps[:],
                            KT[h * 64:h * 64 + 64, kc * 128:(kc + 1) * 128],
                            QT[h * 64:h * 64 + 64, j * 512:(j + 1) * 512],
                            start=True, stop=True,
                        )
                        if kc >= 4 * j:
                            nc.vector.tensor_add(sps[:], sps[:], masks[:, kc - 4 * j, :])
                        pT = ppool.tile([128, 512], F32R, tag="pT")
                        nc.scalar.activation(out=pT[:], in_=sps[:], func=EXP, scale=0.125)
                        nc.tensor.matmul(
                            oaug[h][:], vtil[:, h, kc, :], pT[:],
                            start=(kc == 0), stop=(kc == nkc - 1),
                        )

                # --- D: normalize + build O^T ---
                onorm = [npool.tile([128, 128], F32R, tag="onorm", name=f"onorm{b}_{j}_{q}") for q in range(4)]
                for h in range(HPC):
                    osb = dpool.tile([128, 512], F32, tag="osb")
                    nc.scalar.copy(out=osb[:], in_=oaug[h][:])
                    for qc in range(4):
                        trp = psA.tile([128, 128], F32, tag="a")
                        nc.tensor.transpose(trp[:], osb[:, qc * 128:(qc + 1) * 128], ident[:])
                        recip = spool.tile([128, 1], F32, tag="recip")
                        nc.vector.reciprocal(recip[:], trp[:, 64:65])
                        nc.vector.tensor_scalar_mul(
                            onorm[qc][:, h * 64:h * 64 + 64], trp[:, 0:64], recip[:]
                        )
                for qc in range(4):
                    otp = psA.tile([128, 128], F32R, tag="a")
                    nc.tensor.transpose(otp[:], onorm[qc][:], identr[:])
                    nc.any.tensor_copy(
                        out=OT[:, j * 512 + qc * 128:j * 512 + (qc + 1) * 128], in_=otp[:]
                    )

            # --- E: projection (partial y) ---
            for tch in range(16):
                for co in range(2):
                    yp = psA.tile([128, 512], F32, tag="a")
                    nc.tensor.matmul(
                        yp[:],
                        OT[:, tch * 128:(tch + 1) * 128],
                        wp_sb[:, co * 512:(co + 1) * 512],
                        start=True, stop=True,
                    )
                    ysb = ypool.tile([128, 512], F32, tag="ysb")
                    nc.any.tensor_copy(out=ysb[:], in_=yp[:])
                    nc.sync.dma_start(
                        out=y_out[b * T + tch * 128: b * T + (tch + 1) * 128,
                                  co * 512:(co + 1) * 512],
                        in_=ysb[:],
                    )

    nc.compile()
    return nc


def get_nc():
    if "nc" not in _CACHE:
        _CACHE["nc"] = _build()
    return _CACHE["nc"]


def make_in_maps(x, Wqkv, Wproj):
    x = np.asarray(x, dtype=np.float32)
    Wqkv = np.asarray(Wqkv, dtype=np.float32)
    Wproj = np.asarray(Wproj, dtype=np.float32)
    xT = np.ascontiguousarray(x.reshape(BT, C).T)
    in_maps = []
    for c in range(N_CORES):
        h0, h1 = HPC * c, HPC * c + 1
        cols = []
        for g in range(3):  # q, k, v
            for h in (h0, h1):
                cols.append(Wqkv[:, g * C + h * HD:g * C + (h + 1) * HD])
        wq_shard = np.ascontiguousarray(np.concatenate(cols, axis=1))
        wp_shard = np.ascontiguousarray(Wproj[c * 128:(c + 1) * 128, :])
        in_maps.append({"xT": xT, "wq": wq_shard, "wp": wp_shard})
    return in_maps


def assemble(results):
    y = np.zeros((BT, C), dtype=np.float32)
    for r in results:
        y += r["y_part"]
    y = y.reshape(B, T, C)
    # kT/vT: per core [B, 2*64, T] -> k [B, NH, T, HD]
    kT = np.stack([r["kT"] for r in results], axis=0)  # [8, B, 128, T]
    vT = np.stack([r["vT"] for r in results], axis=0)
    def unshard(aT):
        a = aT.reshape(N_CORES, B, HPC, HD, T)          # [8, B, 2, 64, T]
        a = a.transpose(1, 0, 2, 4, 3)                  # [B, 8, 2, T, 64]
        return np.ascontiguousarray(a.reshape(B, NH, T, HD))
    return y, unshard(kT), unshard(vT)


def kernel(x, Wqkv, Wproj):
    nc = get_nc()
    in_maps = make_in_maps(x, Wqkv, Wproj)
    res = run_bass_kernel_spmd(nc, in_maps, list(range(N_CORES)))
    return assemble(res.results)


if __name__ == "__main__":
    rng = np.random.default_rng(0)
    x = rng.standard_normal((B, T, C), dtype=np.float32)
    Wqkv = (rng.standard_normal((C, 3 * C)) * 0.02).astype(np.float32)
    Wproj = (rng.standard_normal((C, C)) * 0.02).astype(np.float32)
    y, k, v = kernel(x, Wqkv, Wproj)
    print("shapes:", y.shape, k.shape, v.shape)


# revision 9
# speedup vs baseline: 1.1394x; 1.1394x over previous
"""Causal self-attention (B=4, T=2048, C=1024, nh=16) on 8 Trainium2 NeuronCores.

Sharding: tensor-parallel over heads (2 heads/core). Each core receives:
  - xT:  [1024, 8192]  transposed activations (replicated)
  - wq:  [1024, 384]   Wqkv columns for its 2 heads, ordered [q0|q1|k0|k1|v0|v1]
  - wp:  [128, 1024]   Wproj rows for its 2 heads' channels
and produces y_part (full-shape partial, host-summed), plus its kT/vT shards.

Device pipeline per batch b:
  A) qkvT = wq_shard.T @ xT  -> Q^T,K^T,V^T stacked [128, 2048] (f32r)
  B) V~ = PE-transpose(V^T) padded to [128(k),128] with a ones column (rowsum trick)
  C) per 512-q block j: S^T[k,q] = K^T_stat.T @ Q^T_mov; causal mask on diagonal
     tiles; P^T = exp(S^T/8) (ACT, f32r); O_aug^T[65,q] += V~^T @ P^T
  D) transpose O_aug -> divide rows by rowsum col -> transpose back -> O^T
  E) y_part[t, :] = O^T_stat.T @ Wproj_mov  (single 128-contraction)

All matmuls run in float32r (1 cyc/row at N=512; ~1.6e-4 rel err).
"""

import numpy as np
from contextlib import ExitStack

import concourse.bass as bass
import concourse.tile as tile
from concourse import bacc, mybir
from concourse.bass_utils import run_bass_kernel_spmd
from concourse.masks import make_identity

F32 = mybir.dt.float32
F32R = mybir.dt.float32r
EXP = mybir.ActivationFunctionType.Exp

N_CORES = 8
B, T, C = 4, 2048, 1024
NH, HD = 16, 64
HPC = NH // N_CORES          # heads per core = 2
BT = B * T                   # 8192

_CACHE = {}


def _build():
    nc = bacc.Bacc("TRN2", target_bir_lowering=False, debug=False, num_devices=N_CORES)
    xT = nc.dram_tensor("xT", [C, BT], F32, kind="ExternalInput").ap()
    wq = nc.dram_tensor("wq", [C, 3 * HPC * HD], F32, kind="ExternalInput").ap()
    wp = nc.dram_tensor("wp", [HPC * HD, C], F32, kind="ExternalInput").ap()
    y_out = nc.dram_tensor("y_part", [BT, C], F32, kind="ExternalOutput").ap()
    kT_out = nc.dram_tensor("kT", [B, HPC * HD, T], F32, kind="ExternalOutput").ap()
    vT_out = nc.dram_tensor("vT", [B, HPC * HD, T], F32, kind="ExternalOutput").ap()

    with tile.TileContext(nc) as tc, ExitStack() as ctx:
        const = ctx.enter_context(tc.tile_pool(name="const", bufs=1))
        bpool = ctx.enter_context(tc.tile_pool(name="bpool", bufs=2))
        xpool = ctx.enter_context(tc.tile_pool(name="xpool", bufs=2))
        ppool = ctx.enter_context(tc.tile_pool(name="ppool", bufs=4))
        spool = ctx.enter_context(tc.tile_pool(name="spool", bufs=8))
        ypool = ctx.enter_context(tc.tile_pool(name="ypool", bufs=3))
        psA = ctx.enter_context(tc.tile_pool(name="psA", bufs=2, space="PSUM"))
        psS = ctx.enter_context(tc.tile_pool(name="psS", bufs=3, space="PSUM"))
        psO = ctx.enter_context(tc.tile_pool(name="psO", bufs=3, space="PSUM"))

        # --- constants ---
        ident = const.tile([128, 128], F32)
        make_identity(nc, ident[:])
        identr = const.tile([128, 128], F32R)
        nc.scalar.copy(out=identr[:], in_=ident[:])
        # additive causal masks for the 4 diagonal offsets of S^T [k=128, q=512]
        masks = const.tile([128, 4, 512], F32)
        for o in range(4):
            nc.gpsimd.memset(masks[:, o, :], 0.0)
            nc.gpsimd.affine_select(
                out=masks[:, o, :], in_=masks[:, o, :],
                compare_op=mybir.AluOpType.is_ge,
                fill=-1e10, base=-128 * o,
                pattern=[[1, 512]], channel_multiplier=-1,
            )
        # pad block for V~ columns 64..127: col 64 = 1 (rowsum), rest 0
        padcol = const.tile([128, 64], F32)
        nc.any.memset(padcol[:], 0.0)
        nc.any.memset(padcol[:, 0:1], 1.0)
        # ones row for broadcasting rowsum reciprocal across 64 partitions
        ones_f32 = const.tile([1, 64], F32)
        nc.any.memset(ones_f32[:], 1.0)
        ones64 = const.tile([1, 64], F32R)
        nc.scalar.copy(out=ones64[:], in_=ones_f32[:])
        # weights (DMA straight into f32r tiles)
        wq_sb = const.tile([128, 8, 3 * 128], F32R)
        for kc in range(8):
            nc.sync.dma_start(out=wq_sb[:, kc, :], in_=wq[kc * 128:(kc + 1) * 128, :].bitcast(F32R))
        wp_sb = const.tile([128, C], F32R)
        nc.sync.dma_start(out=wp_sb[:], in_=wp[:].bitcast(F32R))

        for b in range(B):
            QT = bpool.tile([128, T], F32R, tag="QT")
            KT = bpool.tile([128, T], F32R, tag="KT")
            VT = bpool.tile([128, T], F32R, tag="VT")
            OT = bpool.tile([128, T], F32R, tag="OT")
            vtil = bpool.tile([128, HPC, 16, 128], F32R, tag="vtil")

            # --- A: qkvT for this batch ---
            for tb in range(4):
                t0 = b * T + tb * 512
                xt = xpool.tile([128, 8, 512], F32R, tag="xt")
                for kc in range(8):
                    nc.sync.dma_start(
                        out=xt[:, kc, :],
                        in_=xT[kc * 128:(kc + 1) * 128, t0:t0 + 512].bitcast(F32R),
                    )
                for g, dest in enumerate((QT, KT, VT)):
                    pq = psA.tile([128, 512], F32, tag="a")
                    for kc in range(8):
                        nc.tensor.matmul(
                            pq[:], wq_sb[:, kc, g * 128:(g + 1) * 128], xt[:, kc, :],
                            start=(kc == 0), stop=(kc == 7),
                        )
                    nc.any.tensor_copy(out=dest[:, tb * 512:tb * 512 + 512], in_=pq[:])
                # V~ build for this tb's 4 token-chunks (interleaved with stage-A
                # matmuls so the PE-transposes never cluster long enough to
                # re-throttle the HAM clock gate)
                for tc_ in range(4 * tb, 4 * tb + 4):
                    pt = psA.tile([128, 128], F32R, tag="a")
                    nc.tensor.transpose(pt[:], VT[:, tc_ * 128:(tc_ + 1) * 128], identr[:])
                    for h in range(HPC):
                        nc.any.tensor_copy(out=vtil[:, h, tc_, 0:64], in_=pt[:, h * 64:h * 64 + 64])
                        nc.scalar.copy(out=vtil[:, h, tc_, 64:128], in_=padcol[:])
            nc.sync.dma_start(out=kT_out[b], in_=KT[:].bitcast(F32))
            nc.sync.dma_start(out=vT_out[b], in_=VT[:].bitcast(F32))

            # --- C: attention per 512-q block ---
            for j in range(4):
                oaug = [psO.tile([128, 512], F32, tag="o", name=f"oaug{b}_{j}_{h}") for h in range(HPC)]
                nkc = 4 * j + 4
                for kc in range(nkc):
                    # emit both heads' S matmuls adjacently: base partitions 0/64
                    # map to PE row-groups (tile_position) so the pair runs
                    # concurrently on the two array halves (~2x)
                    spss = []
                    for h in range(HPC):
                        sps = psS.tile([128, 512], F32, tag="s", name=f"sps{b}_{j}_{kc}_{h}")
                        nc.tensor.matmul(
                            sps[:],
                            KT[h * 64:h * 64 + 64, kc * 128:(kc + 1) * 128],
                            QT[h * 64:h * 64 + 64, j * 512:(j + 1) * 512],
                            start=True, stop=True,
                        )
                        spss.append(sps)
                    pTs = []
                    for h in range(HPC):
                        if kc >= 4 * j:
                            nc.vector.tensor_add(spss[h][:], spss[h][:], masks[:, kc - 4 * j, :])
                        pT = ppool.tile([128, 512], F32R, tag="pT", name=f"pT{b}_{j}_{kc}_{h}")
                        nc.scalar.activation(out=pT[:], in_=spss[h][:], func=EXP, scale=0.125)
                        pTs.append(pT)
                    for h in range(HPC):
                        nc.tensor.matmul(
                            oaug[h][:], vtil[:, h, kc, :], pTs[h][:],
                            start=(kc == 0), stop=(kc == nkc - 1),
                        )

                # --- D: normalize rows 0..63 of O_aug^T by the rowsum in row 64,
                # via reciprocal + ones-matmul partition-broadcast + one DVE mul ---
                for h in range(HPC):
                    rs = spool.tile([1, 512], F32R, tag="rs", name=f"rs{b}_{j}_{h}")
                    with nc.allow_low_precision(reason="f32r reciprocal of rowsum"):
                        nc.vector.reciprocal(rs[:], oaug[h][64:65, :])
                    bc = psO.tile([64, 512], F32, tag="o", name=f"bc{b}_{j}_{h}")
                    nc.tensor.matmul(bc[:], ones64[:], rs[:], start=True, stop=True)
                    bcs = spool.tile([64, 512], F32, tag="bcs", name=f"bcs{b}_{j}_{h}")
                    nc.scalar.copy(out=bcs[:], in_=bc[:])
                    nc.vector.tensor_mul(
                        OT[h * 64:h * 64 + 64, j * 512:(j + 1) * 512],
                        oaug[h][0:64, :],
                        bcs[:],
                    )

            # --- E: projection (partial y) ---
            for tch in range(16):
                for co in range(2):
                    yp = psA.tile([128, 512], F32, tag="a")
                    nc.tensor.matmul(
                        yp[:],
                        OT[:, tch * 128:(tch + 1) * 128],
                        wp_sb[:, co * 512:(co + 1) * 512],
                        start=True, stop=True,
                    )
                    ysb = ypool.tile([128, 512], F32, tag="ysb")
                    nc.any.tensor_copy(out=ysb[:], in_=yp[:])
                    nc.sync.dma_start(
                        out=y_out[b * T + tch * 128: b * T + (tch + 1) * 128,
                                  co * 512:(co + 1) * 512],
                        in_=ysb[:],
                    )

    nc.compile()
    return nc


def get_nc():
    if "nc" not in _CACHE:
        _CACHE["nc"] = _build()
    return _CACHE["nc"]


def make_in_maps(x, Wqkv, Wproj):
    x = np.asarray(x, dtype=np.float32)
    Wqkv = np.asarray(Wqkv, dtype=np.float32)
    Wproj = np.asarray(Wproj, dtype=np.float32)
    xT = np.ascontiguousarray(x.reshape(BT, C).T)
    in_maps = []
    for c in range(N_CORES):
        h0, h1 = HPC * c, HPC * c + 1
        cols = []
        for g in range(3):  # q, k, v
            for h in (h0, h1):
                cols.append(Wqkv[:, g * C + h * HD:g * C + (h + 1) * HD])
        wq_shard = np.ascontiguousarray(np.concatenate(cols, axis=1))
        wp_shard = np.ascontiguousarray(Wproj[c * 128:(c + 1) * 128, :])
        in_maps.append({"xT": xT, "wq": wq_shard, "wp": wp_shard})
    return in_maps


def assemble(results):
    y = np.zeros((BT, C), dtype=np.float32)
    for r in results:
        y += r["y_part"]
    y = y.reshape(B, T, C)
    # kT/vT: per core [B, 2*64, T] -> k [B, NH, T, HD]
    kT = np.stack([r["kT"] for r in results], axis=0)  # [8, B, 128, T]
    vT = np.stack([r["vT"] for r in results], axis=0)
    def unshard(aT):
        a = aT.reshape(N_CORES, B, HPC, HD, T)          # [8, B, 2, 64, T]
        a = a.transpose(1, 0, 2, 4, 3)                  # [B, 8, 2, T, 64]
        return np.ascontiguousarray(a.reshape(B, NH, T, HD))
    return y, unshard(kT), unshard(vT)


def kernel(x, Wqkv, Wproj):
    nc = get_nc()
    in_maps = make_in_maps(x, Wqkv, Wproj)
    res = run_bass_kernel_spmd(nc, in_maps, list(range(N_CORES)))
    return assemble(res.results)


if __name__ == "__main__":
    rng = np.random.default_rng(0)
    x = rng.standard_normal((B, T, C), dtype=np.float32)
    Wqkv = (rng.standard_normal((C, 3 * C)) * 0.02).astype(np.float32)
    Wproj = (rng.standard_normal((C, C)) * 0.02).astype(np.float32)
    y, k, v = kernel(x, Wqkv, Wproj)
    print("shapes:", y.shape, k.shape, v.shape)
